# revision 1
# baseline (speedup 1.0000x reference)
"""Trainium2 Bass kernel for nn_CELoss_Marginal_Smooth (CE loss with marginal
attention smoothing) on 8 NeuronCores.

Strategy
--------
loss = -mean_i[ (1-w2_i)*x[i,t_i] + w2_i*S_i - (1+11*w2_i)*lse_i ]
  where S_i = sum_c x[i,c], lse_i = log(sum_c exp(x[i,c])), and
  w2_i = (1-ALPHA)*att(t_i) takes one of 12 per-class values.

The host shards rows across 8 cores AND groups rows by target class inside
each core's shard (the loss is permutation-invariant, so row order is a
sharding/layout choice). Each (partition, class) cell is padded with zero
rows to a uniform count qpc, so on-device every class occupies a static
rectangular block [128, qpc, 12]. All target-dependent selection then
disappears:
  - sum_i w2_i * S_i            -> PE ones-matmul over the class block with
                                   the class weight folded into the
                                   stationary vector
  - sum_i (1-w2_i) * x[i,t_i]   -> same, over the block's own-class column
  - sum_i wl_i * lse_i          -> ACT ln(sum-exp) with per-instruction
                                   accumulate, PE-contracted over partitions
  - sumexp                      -> DVE pairwise-add tree over exp(x)
Each pad row contributes exactly -wl_c*ln(12); corrected on the host from
known pad counts. The host combines the 8 partial sums (the unshard step).
"""
import sys

if "/opt/trn_rl_repo" not in sys.path:
    sys.path.insert(0, "/opt/trn_rl_repo")

import math
from contextlib import ExitStack

import numpy as np

import concourse.bass as bass
import concourse.tile as tile
from concourse import bacc, mybir
from concourse.bass_utils import run_bass_kernel_spmd
from concourse.tile_rust import add_dep_helper

C = 12
P = 128
NCORES = 8
ALPHA = 0.6
GROUP = 2          # classes whose E tiles share one DVE tree pass
MM_CHUNK = 512     # moving free-dim per rect matmul

_F32 = mybir.dt.float32
_F32R = mybir.dt.float32r
_AF = mybir.ActivationFunctionType


def _att_values():
    i = np.arange(C)
    r, c = i // 4, i % 4
    up, dn = (r - 1 >= 0), (r + 1 <= 2)
    lf, rt = (c - 1 >= 0), (c + 1 <= 3)
    cnt = (up.astype(np.int32) + dn + lf + rt
           + (up & lf) + (up & rt) + (dn & lf) + (dn & rt))
    return 1.0 / cnt


def _weights():
    att = _att_values()
    w2 = (1.0 - ALPHA) * att          # weight of S_i
    w1 = 1.0 - w2                     # weight of x[i, t_i]
    wl = 1.0 + 11.0 * w2              # weight of lse_i (negated on device)
    return w2, w1, wl


def _build(qpc: int, ablate: frozenset = frozenset(), fp32mm: bool = False):
    """Build + finalize the per-core Bass program for a given qpc.

    `ablate` is a timing-experiment knob ({"tree","exp","mm","ln"}): named
    stages are skipped, producing a wrong but schedulable program.
    `fp32mm` loads x via HWDGE as fp32 and runs plain-fp32 matmuls instead
    of the SWDGE fp32r-cast path.
    """
    fpc = qpc * C                     # free elements per class block
    nc = bacc.Bacc("TRN2", target_bir_lowering=False, debug=False,
                   num_devices=NCORES)
    x = nc.declare_dram_parameter("x", [P, C * fpc], _F32, isOutput=False)
    wt = nc.declare_dram_parameter("wt", [P, 3 * C], _F32, isOutput=False)
    out = nc.declare_dram_parameter("out", [1, 1], _F32, isOutput=True)

    n_groups = C // GROUP
    with tile.TileContext(nc) as tc, ExitStack() as ctx:
        xp = ctx.enter_context(tc.tile_pool(name="xp", bufs=3))
        ep = ctx.enter_context(tc.tile_pool(name="ep", bufs=2))
        tp = ctx.enter_context(tc.tile_pool(name="tp", bufs=2))
        sp = ctx.enter_context(tc.tile_pool(name="sp", bufs=1))
        pp = ctx.enter_context(tc.tile_pool(name="pp", bufs=1, space="PSUM"))

        # fp32r copy feeds the PE (1 cyc/row vs 4 for fp32); fp32 copy feeds
        # the lse matmuls whose lhsT (lacc) is fp32
        x_dt = _F32 if fp32mm else _F32R
        wtile = sp.tile([P, 3 * C], _F32)
        nc.sync.dma_start(wtile[:], wt[:])
        if fp32mm:
            wtile_r = wtile
        else:
            wtile_r = sp.tile([P, 3 * C], _F32R)
            nc.gpsimd.dma_start(wtile_r[:], wt[:])
        lacc = sp.tile([P, C], _F32)
        sebuf = sp.tile([P, C * qpc], _F32)
        ps = pp.tile([1, MM_CHUNK], _F32)

        first_mm = True
        for g in range(n_groups):
            xts = []
            for u in range(GROUP):
                c = g * GROUP + u
                # SWDGE load casts fp32 -> fp32r in the DMA datapath, so the
                # PE gets pre-rounded operands for free
                xt = xp.tile([P, fpc], x_dt, tag="x")
                if fp32mm:
                    nc.sync.dma_start(xt[:], x[:, c * fpc:(c + 1) * fpc])
                else:
                    nc.gpsimd.dma_start(xt[:], x[:, c * fpc:(c + 1) * fpc])
                xts.append(xt)

            # exp into the group's E buffer (per class instruction)
            et = ep.tile([P, GROUP * qpc, C], _F32, tag="e")
            for u in range(GROUP) if "exp" not in ablate else []:
                last_exp = nc.scalar.activation(
                    et[:, u * qpc:(u + 1) * qpc, :],
                    xts[u][:].bitcast(_F32).rearrange("p (q c) -> p q c", c=C),
                    _AF.Exp,
                )

            # pairwise-add tree: sumexp over the class dim
            gq = GROUP * qpc
            if "tree" not in ablate:
                t6 = tp.tile([P, gq, 6], _F32, tag="t6")
                nc.vector.tensor_add(t6[:], et[:, :, 0:6], et[:, :, 6:12])
                t3 = tp.tile([P, gq, 3], _F32, tag="t3")
                nc.vector.tensor_add(t3[:], t6[:, :, 0:3], t6[:, :, 3:6])
                t1 = tp.tile([P, gq, 1], _F32, tag="t1")
                nc.vector.tensor_add(t1[:], t3[:, :, 0:1], t3[:, :, 1:2])
                # sumexp lands in the persistent per-class buffer; ln is
                # deferred past the loop so the ACT stream is all-Exp then
                # all-Ln (2 table loads instead of one per switch)
                seslice = sebuf[:, g * gq:(g + 1) * gq]
                nc.vector.tensor_add(seslice, t1[:], t3[:, :, 2:3])

            for u in range(GROUP) if "mm" not in ablate else []:
                c = g * GROUP + u
                # PE: w2_c * (sum of the whole class block), accumulated
                xr = xts[u][:]
                w2v = wtile_r[:, c:c + 1]
                for i in range(0, fpc, MM_CHUNK):
                    w = min(MM_CHUNK, fpc - i)
                    nc.tensor.matmul(ps[:, 0:w], lhsT=w2v, rhs=xr[:, i:i + w],
                                     start=first_mm, stop=False)
                    first_mm = False
                # PE: (1-w2_c) * (sum of the own-class column)
                xcol = xts[u][:].rearrange("p (q c) -> p q c", c=C)[:, :, c]
                nc.tensor.matmul(
                    ps[:, 0:qpc],
                    lhsT=wtile_r[:, C + c:C + c + 1],
                    rhs=xcol,
                    start=False, stop=False,
                )

        # deferred: lse = ln(sumexp) with per-class accumulate, then
        # ps[0,0] += sum_p lacc[p,c] * (-wl_c)
        lsed = sp.tile([P, qpc], _F32)
        for c in range(C) if "ln" not in ablate else []:
            ln_inst = nc.scalar.activation(
                lsed[:],
                sebuf[:, c * qpc:(c + 1) * qpc],
                _AF.Ln,
                accum_out=lacc[:, c:c + 1],
            )
            # same-engine ordering constraint: keep the ACT stream all-Exp
            # then all-Ln so only two activation-table loads are emitted
            if "exp" not in ablate:
                add_dep_helper(ln_inst.ins, last_exp.ins, False,
                               "ln after all exps (act table batching)")
        for c in range(C) if "mm" not in ablate else []:
            nc.tensor.matmul(ps[:, 0:1], lhsT=lacc[:, c:c + 1],
                             rhs=wtile[:, 2 * C + c:2 * C + c + 1],
                             start=False, stop=(c == C - 1))

        fin = sp.tile([1, 1], _F32)
        nc.vector.tensor_reduce(fin[:], ps[0:1, :], axis=mybir.AxisListType.X,
                                op=mybir.AluOpType.add)
        nc.sync.dma_start(out[:], fin[:])
    nc.finalize()
    return nc


_PROG_CACHE: dict = {}
_LAST_IN_MAPS = None


def _program(qpc: int):
    if qpc not in _PROG_CACHE:
        _PROG_CACHE[qpc] = _build(qpc)
    return _PROG_CACHE[qpc]


def kernel(outputs: np.ndarray, targets: np.ndarray) -> np.ndarray:
    x = np.ascontiguousarray(np.asarray(outputs, dtype=np.float32))
    t = np.asarray(targets).astype(np.int64, copy=False).ravel()
    B = x.shape[0]
    assert x.shape == (B, C)

    counts = np.bincount(t, minlength=C)
    slots = NCORES * P
    # uniform per-(partition, class) row count; multiple of 32 keeps every
    # class block 128-float aligned in the free dim
    qpc = max(352, 32 * math.ceil(counts.max() / (slots * 32)))

    # class-major index layout: A[k, p, c*qpc + j] = global row (or -1 pad)
    A = np.full((C, slots * qpc), -1, dtype=np.int64)
    order = np.argsort(t, kind="stable")
    bounds = np.concatenate(([0], np.cumsum(counts)))
    for c in range(C):
        A[c, :counts[c]] = order[bounds[c]:bounds[c + 1]]
    A = A.reshape(C, slots, qpc).transpose(1, 0, 2).reshape(NCORES, P, C * qpc)

    w2, w1, wl = _weights()
    wtab = np.empty((P, 3 * C), np.float32)
    wtab[:, 0:C] = w2
    wtab[:, C:2 * C] = w1
    wtab[:, 2 * C:3 * C] = -wl

    in_maps = []
    for k in range(NCORES):
        idx = A[k]
        g = x[idx.clip(min=0)]                    # [P, C*qpc, C]
        g[idx < 0] = 0.0
        in_maps.append({"x": np.ascontiguousarray(g.reshape(P, -1)),
                        "wt": wtab})

    nc = _program(qpc)
    global _LAST_IN_MAPS
    _LAST_IN_MAPS = in_maps
    res = run_bass_kernel_spmd(nc, in_maps, list(range(NCORES)))

    partial = sum(float(np.asarray(res.results[k]["out"]).reshape(-1)[0])
                  for k in range(NCORES))
    npad = qpc * slots - counts
    padcorr = float((npad * wl).sum() * math.log(12.0))
    loss = -(partial + padcorr) / B
    return np.float32(loss)


if __name__ == "__main__":
    rng = np.random.default_rng(1)
    Bs = 4194304
    xs = rng.standard_normal((Bs, C)).astype(np.float32)
    ts = rng.integers(0, C, size=Bs).astype(np.int64)
    print("loss:", kernel(xs, ts))



# revision 5
# speedup vs baseline: 1.5018x; 1.5018x over previous
"""Trainium2 Bass kernel for nn_CELoss_Marginal_Smooth (CE loss with marginal
attention smoothing) on 8 NeuronCores.

Strategy
--------
loss = -mean_i[ (1-w2_i)*x[i,t_i] + w2_i*S_i - (1+11*w2_i)*lse_i ]
  where S_i = sum_c x[i,c], lse_i = log(sum_c exp(x[i,c])), and
  w2_i = (1-ALPHA)*att(t_i) takes one of 12 per-class values.

The host shards rows across 8 cores AND groups rows by target class inside
each core's shard (the loss is permutation-invariant, so row order is a
sharding/layout choice). Each (partition, class) cell is padded with zero
rows to a uniform count qpc. The staged per-core buffer is bf16 and
logit-plane-major within each class block: X[p, c, j, q] = x[row(p,c,q), j],
so every device-side operand is a contiguous bf16 slice:
  - exp            -> one ACT instruction per class group (the bottleneck:
                      1 elem/cycle/partition at 1.2 GHz, dtype-independent)
  - sumexp         -> DVE pairwise-add tree over contiguous bf16 planes
                      (2x packed mode; class planes adjacent in the free dim)
  - per-class sums -> PE ones-matmuls into per-class PSUM rows: S_c (whole
                      block), XT_c (own-logit plane), L_c (lse plane)
  - lse            -> one deferred ACT ln over the packed sumexp buffer
The device emits 12x3 raw per-class partials; the host applies the exact
fp64 class weights, corrects the known pad-row contribution (each pad row
adds exactly ln(12) to its L_c), and combines the 8 cores.
"""
import sys

if "/opt/trn_rl_repo" not in sys.path:
    sys.path.insert(0, "/opt/trn_rl_repo")

import math
from contextlib import ExitStack

import numpy as np

import concourse.bass as bass
import concourse.tile as tile
from concourse import bacc, mybir
from concourse.bass_utils import run_bass_kernel_spmd
from concourse.tile_rust import add_dep_helper

C = 12
P = 128
NCORES = 8
ALPHA = 0.6
MM_CHUNK = 512     # moving free-dim per rect matmul (PSUM bank width)

_F32 = mybir.dt.float32
_BF16 = mybir.dt.bfloat16
_AF = mybir.ActivationFunctionType

# group layout: (first class, n classes). The first two groups are single
# classes so the first exp only waits on a 1-class DMA.
_GROUPS = [(0, 1), (1, 1), (2, 2), (4, 2), (6, 2), (8, 2), (10, 2)]


def _att_values():
    i = np.arange(C)
    r, c = i // 4, i % 4
    up, dn = (r - 1 >= 0), (r + 1 <= 2)
    lf, rt = (c - 1 >= 0), (c + 1 <= 3)
    cnt = (up.astype(np.int32) + dn + lf + rt
           + (up & lf) + (up & rt) + (dn & lf) + (dn & rt))
    return 1.0 / cnt


def _weights():
    att = _att_values()
    w2 = (1.0 - ALPHA) * att          # weight of S_i
    w1 = 1.0 - w2                     # weight of x[i, t_i]
    wl = 1.0 + 11.0 * w2              # weight of lse_i
    return w2, w1, wl


def _build(qpc: int):
    """Build + finalize the per-core Bass program for a given qpc."""
    fp = qpc * C                      # free elements per class block
    nc = bacc.Bacc("TRN2", target_bir_lowering=False, debug=False,
                   num_devices=NCORES)
    x = nc.declare_dram_parameter("x", [P, C * fp], _BF16, isOutput=False)
    out = nc.declare_dram_parameter("out", [C, 3], _F32, isOutput=True)

    with tile.TileContext(nc) as tc, ExitStack() as ctx:
        xp = ctx.enter_context(tc.tile_pool(name="xp", bufs=3))
        ep = ctx.enter_context(tc.tile_pool(name="ep", bufs=2))
        tp = ctx.enter_context(tc.tile_pool(name="tp", bufs=2))
        sp = ctx.enter_context(tc.tile_pool(name="sp", bufs=1))
        pp = ctx.enter_context(tc.tile_pool(name="pp", bufs=1, space="PSUM"))

        # matmul outputs must land at PSUM base partition 0, so class c's
        # partial sums are routed to PSUM row c via a one-hot stationary:
        # oneh[:, c*C + c] = 1, rest 0 -> out row c = column sums, rows
        # m != c accumulate zeros
        oneh = sp.tile([P, C * C], _BF16)
        nc.vector.memset(oneh[:], 0.0)
        ohv = oneh[:].rearrange("p (a b) -> p a b", a=C)
        for c in range(C):
            nc.vector.memset(ohv[:, c, c:c + 1], 1.0)
        sebuf = sp.tile([P, C * qpc], _BF16)
        lsed = sp.tile([P, C * qpc], _BF16)
        ps_s = pp.tile([C, MM_CHUNK], _F32)
        ps_xt = pp.tile([C, MM_CHUNK], _F32)
        ps_l = pp.tile([C, MM_CHUNK], _F32)

        last_exp = None
        for c0, ng in _GROUPS:
            gf = ng * fp
            xt = xp.tile([P, gf], _BF16, tag="x")
            nc.sync.dma_start(xt[:], x[:, c0 * fp:c0 * fp + gf])

            et = ep.tile([P, gf], _BF16, tag="e")
            last_exp = nc.scalar.activation(et[:], xt[:], _AF.Exp)

            # pairwise-add tree over the 12 logit planes of each class in
            # the group; all operands are contiguous bf16 runs of qpc (2x
            # packed DVE mode)
            ev = et[:].rearrange("p (t j q) -> p t j q", t=ng, j=C)
            t6 = tp.tile([P, ng, 6, qpc], _BF16, tag="t6")
            nc.vector.tensor_add(t6[:], ev[:, :, 0:6], ev[:, :, 6:12])
            t3 = tp.tile([P, ng, 3, qpc], _BF16, tag="t3")
            nc.vector.tensor_add(t3[:], t6[:, :, 0:3], t6[:, :, 3:6])
            t1 = tp.tile([P, ng, 1, qpc], _BF16, tag="t1")
            nc.vector.tensor_add(t1[:], t3[:, :, 0:1], t3[:, :, 1:2])
            sev = sebuf[:, c0 * qpc:(c0 + ng) * qpc].rearrange(
                "p (t j q) -> p t j q", t=ng, j=1)
            nc.vector.tensor_add(sev, t1[:], t3[:, :, 2:3])

            for u in range(ng):
                c = c0 + u
                lh = oneh[:, c * C:(c + 1) * C]
                # S_c: sum of the whole class block, accumulated column-wise
                xflat = xt[:, u * fp:(u + 1) * fp]
                for i in range(0, fp, MM_CHUNK):
                    w = min(MM_CHUNK, fp - i)
                    nc.tensor.matmul(ps_s[:, 0:w], lhsT=lh,
                                     rhs=xflat[:, i:i + w],
                                     start=(c == 0 and i == 0),
                                     stop=(c == C - 1 and i + MM_CHUNK >= fp))
                # XT_c: sum of the own-logit plane
                xplane = xt[:, u * fp + c * qpc:u * fp + (c + 1) * qpc]
                nc.tensor.matmul(ps_xt[:, 0:qpc], lhsT=lh, rhs=xplane,
                                 start=(c == 0), stop=(c == C - 1))

        # deferred: lse = ln(sumexp) in one ACT pass (same-engine ordering
        # keeps the ACT stream all-Exp then Ln: 2 activation-table loads)
        ln_inst = nc.scalar.activation(lsed[:], sebuf[:], _AF.Ln)
        add_dep_helper(ln_inst.ins, last_exp.ins, False,
                       "ln after all exps (act table batching)")
        for c in range(C):
            nc.tensor.matmul(ps_l[:, 0:qpc], lhsT=oneh[:, c * C:(c + 1) * C],
                             rhs=lsed[:, c * qpc:(c + 1) * qpc],
                             start=(c == 0), stop=(c == C - 1))

        fin = sp.tile([C, 3], _F32)
        nc.vector.tensor_reduce(fin[:, 0:1], ps_s[:, :],
                                axis=mybir.AxisListType.X,
                                op=mybir.AluOpType.add)
        nc.vector.tensor_reduce(fin[:, 1:2], ps_xt[:, 0:qpc],
                                axis=mybir.AxisListType.X,
                                op=mybir.AluOpType.add)
        nc.vector.tensor_reduce(fin[:, 2:3], ps_l[:, 0:qpc],
                                axis=mybir.AxisListType.X,
                                op=mybir.AluOpType.add)
        nc.sync.dma_start(out[:], fin[:])
    nc.finalize()
    return nc


_PROG_CACHE: dict = {}
_LAST_IN_MAPS = None


def _program(qpc: int):
    if qpc not in _PROG_CACHE:
        _PROG_CACHE[qpc] = _build(qpc)
    return _PROG_CACHE[qpc]


def kernel(outputs: np.ndarray, targets: np.ndarray) -> np.ndarray:
    import ml_dtypes

    bf16 = ml_dtypes.bfloat16
    x = np.asarray(outputs)
    t = np.asarray(targets).astype(np.int64, copy=False).ravel()
    B = x.shape[0]
    assert x.shape == (B, C)

    counts = np.bincount(t, minlength=C)
    slots = NCORES * P
    # uniform per-(partition, class) row count; multiple of 8 keeps every
    # bf16 plane 16-byte aligned in the free dim
    qpc = max(64, 8 * math.ceil(counts.max() / (slots * 8)))

    # class-major index layout: A[c, s, q] = global row (or -1 pad)
    A = np.full((C, slots * qpc), -1, dtype=np.int64)
    order = np.argsort(t, kind="stable")
    bounds = np.concatenate(([0], np.cumsum(counts)))
    for c in range(C):
        A[c, :counts[c]] = order[bounds[c]:bounds[c + 1]]
    A = A.reshape(C, slots, qpc)

    xb = x.astype(bf16)
    in_maps = []
    for k in range(NCORES):
        idx = A[:, k * P:(k + 1) * P, :]          # [C, P, qpc]
        g = xb[idx.clip(min=0)]                   # [C, P, qpc, 12]
        g[idx < 0] = 0
        # plane-major within each class block: [p, c, j, q]
        xk = np.ascontiguousarray(g.transpose(1, 0, 3, 2)).reshape(P, -1)
        in_maps.append({"x": xk})

    nc = _program(qpc)
    global _LAST_IN_MAPS
    _LAST_IN_MAPS = in_maps
    res = run_bass_kernel_spmd(nc, in_maps, list(range(NCORES)))

    acc = np.zeros((C, 3), dtype=np.float64)
    for k in range(NCORES):
        acc += np.asarray(res.results[k]["out"]).astype(np.float64)
    s_c, xt_c, l_c = acc[:, 0], acc[:, 1], acc[:, 2]

    w2, w1, wl = _weights()
    npad = qpc * slots - counts
    l_c = l_c - npad * math.log(12.0)             # pad rows: se = 12 exactly
    partial = (w1 * xt_c + w2 * s_c - wl * l_c).sum()
    loss = -partial / B
    return np.float32(loss)


if __name__ == "__main__":
    rng = np.random.default_rng(1)
    Bs = 4194304
    xs = rng.standard_normal((Bs, C)).astype(np.float32)
    ts = rng.integers(0, C, size=Bs).astype(np.int64)
    print("loss:", kernel(xs, ts))


# revision 17
# speedup vs baseline: 1.5783x; 1.0509x over previous
"""Trainium2 Bass kernel for nn_CELoss_Marginal_Smooth (CE loss with marginal
attention smoothing) on 8 NeuronCores.

Strategy
--------
loss = -mean_i[ (1-w2_i)*x[i,t_i] + w2_i*S_i - (1+11*w2_i)*lse_i ]
  where S_i = sum_c x[i,c], lse_i = log(sum_c exp(x[i,c])), and
  w2_i = (1-ALPHA)*att(t_i) takes one of 12 per-class values.

The host shards rows across 8 cores AND groups rows by target class inside
each core's shard (the loss is permutation-invariant, so row order is a
sharding/layout choice). Each (partition, class) cell is padded with zero
rows to a uniform count qpc. The staged per-core buffer is bf16 and
logit-plane-major within each class block: X[p, c, j, q] = x[row(p,c,q), j],
so every device-side operand is a contiguous bf16 slice:
  - exp            -> one ACT instruction per class group (the bottleneck:
                      1 elem/cycle/partition at 1.2 GHz, dtype-independent)
  - sumexp         -> DVE pairwise-add tree over contiguous bf16 planes
                      (2x packed mode; class planes adjacent in the free dim)
  - per-class sums -> PE ones-matmuls into per-class PSUM rows: S_c (whole
                      block), XT_c (own-logit plane), L_c (lse plane)
  - lse            -> one deferred ACT ln over the packed sumexp buffer
The device emits 12x3 raw per-class partials; the host applies the exact
fp64 class weights, corrects the known pad-row contribution (each pad row
adds exactly ln(12) to its L_c), and combines the 8 cores.
"""
import sys

if "/opt/trn_rl_repo" not in sys.path:
    sys.path.insert(0, "/opt/trn_rl_repo")

import math
from contextlib import ExitStack

import numpy as np

import concourse.bass as bass
import concourse.tile as tile
from concourse import bacc, mybir
from concourse.bass_utils import run_bass_kernel_spmd
from concourse.tile_rust import add_dep_helper

C = 12
P = 128
NCORES = 8
ALPHA = 0.6
MM_CHUNK = 512     # moving free-dim per rect matmul (PSUM bank width)

_F32 = mybir.dt.float32
_BF16 = mybir.dt.bfloat16
_FP8 = mybir.dt.float8e4
_AF = mybir.ActivationFunctionType

# group layout: (first class, n classes). The first two groups are single
# classes so the first exp only waits on a 1-class DMA; the last two are
# single classes to shorten the tail chain (exp -> tree -> ln -> out).
_GROUPS = [(0, 1), (1, 1), (2, 2), (4, 2), (6, 2), (8, 2), (10, 1), (11, 1)]
_HEAD_SPLIT = 3    # planes of class 0 in the first (latency-critical) DMA


def _att_values():
    i = np.arange(C)
    r, c = i // 4, i % 4
    up, dn = (r - 1 >= 0), (r + 1 <= 2)
    lf, rt = (c - 1 >= 0), (c + 1 <= 3)
    cnt = (up.astype(np.int32) + dn + lf + rt
           + (up & lf) + (up & rt) + (dn & lf) + (dn & rt))
    return 1.0 / cnt


def _weights():
    att = _att_values()
    w2 = (1.0 - ALPHA) * att          # weight of S_i
    w1 = 1.0 - w2                     # weight of x[i, t_i]
    wl = 1.0 + 11.0 * w2              # weight of lse_i
    return w2, w1, wl


def _build(qpc: int):
    """Build + finalize the per-core Bass program for a given qpc."""
    fp = qpc * C                      # free elements per class block
    nc = bacc.Bacc("TRN2", target_bir_lowering=False, debug=False,
                   num_devices=NCORES)
    # NOTE: the combined natural_log_exp_and_others activation table set
    # produces wrong Ln results on hardware (probed 2026-08-08); keep the
    # default per-function table selection (exp_and_others + natural_log).
    x = nc.declare_dram_parameter("x", [P, C * fp], _FP8, isOutput=False)
    out = nc.declare_dram_parameter("out", [C, 3], _F32, isOutput=True)

    with tile.TileContext(nc) as tc, ExitStack() as ctx:
        xp = ctx.enter_context(tc.tile_pool(name="xp", bufs=4))
        ep = ctx.enter_context(tc.tile_pool(name="ep", bufs=2))
        tp = ctx.enter_context(tc.tile_pool(name="tp", bufs=2))
        sp = ctx.enter_context(tc.tile_pool(name="sp", bufs=1))
        pp = ctx.enter_context(tc.tile_pool(name="pp", bufs=1, space="PSUM"))

        # matmul outputs must land at PSUM base partition 0, so class c's
        # partial sums are routed to PSUM row c via a one-hot stationary:
        # oneh[:, c*C + c] = 1, rest 0 -> out row c = column sums, rows
        # m != c accumulate zeros. One copy per matmul operand dtype.
        oneh8 = sp.tile([P, C * C], _FP8)
        onehb = sp.tile([P, C * C], _BF16)
        for oh in (oneh8, onehb):
            nc.vector.memset(oh[:], 0.0)
            ohv = oh[:].rearrange("p (a b) -> p a b", a=C)
            for c in range(C):
                nc.vector.memset(ohv[:, c, c:c + 1], 1.0)
        sebuf = sp.tile([P, C * qpc], _BF16)
        lsed = sp.tile([P, C * qpc], _BF16)
        ps_s = pp.tile([C, MM_CHUNK], _F32)
        ps_xt = pp.tile([C, MM_CHUNK], _F32)
        ps_l = pp.tile([C, MM_CHUNK], _F32)

        last_exp = None
        for c0, ng in _GROUPS:
            gf = ng * fp
            xt = xp.tile([P, gf], _FP8, tag="x")
            et = ep.tile([P, gf], _BF16, tag="e")
            if c0 == 0:
                # split the first load so the exp pipeline starts as soon as
                # a small head chunk lands (DMA fixed latency dominates)
                h = _HEAD_SPLIT * qpc
                nc.sync.dma_start(xt[:, 0:h], x[:, 0:h])
                nc.sync.dma_start(xt[:, h:gf], x[:, h:gf])
                nc.scalar.activation(et[:, 0:h], xt[:, 0:h], _AF.Exp)
                last_exp = nc.scalar.activation(et[:, h:gf], xt[:, h:gf],
                                                _AF.Exp)
            else:
                nc.sync.dma_start(xt[:], x[:, c0 * fp:c0 * fp + gf])
                last_exp = nc.scalar.activation(et[:], xt[:], _AF.Exp)

            # pairwise-add tree over the 12 logit planes of each class in
            # the group; all operands are contiguous bf16 runs of qpc (2x
            # packed DVE mode)
            ev = et[:].rearrange("p (t j q) -> p t j q", t=ng, j=C)
            t6 = tp.tile([P, ng, 6, qpc], _BF16, tag="t6")
            nc.vector.tensor_add(t6[:], ev[:, :, 0:6], ev[:, :, 6:12])
            t3 = tp.tile([P, ng, 3, qpc], _BF16, tag="t3")
            nc.vector.tensor_add(t3[:], t6[:, :, 0:3], t6[:, :, 3:6])
            t1 = tp.tile([P, ng, 1, qpc], _BF16, tag="t1")
            nc.vector.tensor_add(t1[:], t3[:, :, 0:1], t3[:, :, 1:2])
            sev = sebuf[:, c0 * qpc:(c0 + ng) * qpc].rearrange(
                "p (t j q) -> p t j q", t=ng, j=1)
            nc.vector.tensor_add(sev, t1[:], t3[:, :, 2:3])

            for u in range(ng):
                c = c0 + u
                lh = oneh8[:, c * C:(c + 1) * C]
                # S_c: sum of the whole class block, accumulated column-wise
                xflat = xt[:, u * fp:(u + 1) * fp]
                for i in range(0, fp, MM_CHUNK):
                    w = min(MM_CHUNK, fp - i)
                    nc.tensor.matmul(ps_s[:, 0:w], lhsT=lh,
                                     rhs=xflat[:, i:i + w],
                                     start=(c == 0 and i == 0),
                                     stop=(c == C - 1 and i + MM_CHUNK >= fp))
                # XT_c: sum of the own-logit plane
                xplane = xt[:, u * fp + c * qpc:u * fp + (c + 1) * qpc]
                nc.tensor.matmul(ps_xt[:, 0:qpc], lhsT=lh, rhs=xplane,
                                 start=(c == 0), stop=(c == C - 1))

        fin = sp.tile([C, 3], _F32)
        # S/XT accumulation groups close with class 11's matmuls, which only
        # depend on the last DMA -- these reduces overlap the ln stream
        nc.vector.tensor_reduce(fin[:, 0:1], ps_s[:, :],
                                axis=mybir.AxisListType.X,
                                op=mybir.AluOpType.add)
        nc.vector.tensor_reduce(fin[:, 1:2], ps_xt[:, 0:qpc],
                                axis=mybir.AxisListType.X,
                                op=mybir.AluOpType.add)

        # deferred: lse = ln(sumexp). Two ACT passes: the bulk (classes
        # 0..9, whose trees completed long before the last exp) and a short
        # tail pass for the last two classes, so the final ln only waits on
        # the last group's tree.
        for lo_c, hi_c in ((0, C - 2), (C - 2, C)):
            lo, hi = lo_c * qpc, hi_c * qpc
            ln_inst = nc.scalar.activation(lsed[:, lo:hi], sebuf[:, lo:hi],
                                           _AF.Ln)
            add_dep_helper(ln_inst.ins, last_exp.ins, False,
                           "ln after all exps (act stream ordering)")
            for c in range(lo_c, hi_c):
                nc.tensor.matmul(ps_l[:, 0:qpc],
                                 lhsT=onehb[:, c * C:(c + 1) * C],
                                 rhs=lsed[:, c * qpc:(c + 1) * qpc],
                                 start=(c == 0), stop=(c == C - 1))
        nc.vector.tensor_reduce(fin[:, 2:3], ps_l[:, 0:qpc],
                                axis=mybir.AxisListType.X,
                                op=mybir.AluOpType.add)
        nc.sync.dma_start(out[:], fin[:])
    nc.finalize()
    return nc


_PROG_CACHE: dict = {}
_LAST_IN_MAPS = None


def _program(qpc: int):
    if qpc not in _PROG_CACHE:
        _PROG_CACHE[qpc] = _build(qpc)
    return _PROG_CACHE[qpc]


def kernel(outputs: np.ndarray, targets: np.ndarray) -> np.ndarray:
    x = np.asarray(outputs)
    t = np.asarray(targets).astype(np.int64, copy=False).ravel()
    B = x.shape[0]
    assert x.shape == (B, C)

    counts = np.bincount(t, minlength=C)
    slots = NCORES * P
    # uniform per-(partition, class) row count; multiple of 8 keeps every
    # staged plane 4-byte aligned in the free dim
    qpc = max(64, 8 * math.ceil(counts.max() / (slots * 8)))

    # class-major index layout: A[c, s, q] = global row (or -1 pad)
    A = np.full((C, slots * qpc), -1, dtype=np.int64)
    order = np.argsort(t, kind="stable")
    bounds = np.concatenate(([0], np.cumsum(counts)))
    for c in range(C):
        A[c, :counts[c]] = order[bounds[c]:bounds[c + 1]]
    A = A.reshape(C, slots, qpc)

    xb = x.astype(mybir.dt.np(_FP8))
    in_maps = []
    for k in range(NCORES):
        idx = A[:, k * P:(k + 1) * P, :]          # [C, P, qpc]
        g = xb[idx.clip(min=0)]                   # [C, P, qpc, 12]
        g[idx < 0] = 0
        # plane-major within each class block: [p, c, j, q]
        xk = np.ascontiguousarray(g.transpose(1, 0, 3, 2)).reshape(P, -1)
        in_maps.append({"x": xk})

    nc = _program(qpc)
    global _LAST_IN_MAPS
    _LAST_IN_MAPS = in_maps

    # guard against transient execution flakes: rerun until two consecutive
    # runs agree on the combined partials
    prev = None
    for _ in range(4):
        res = run_bass_kernel_spmd(nc, in_maps, list(range(NCORES)))
        acc = np.zeros((C, 3), dtype=np.float64)
        for k in range(NCORES):
            acc += np.asarray(res.results[k]["out"]).astype(np.float64)
        if prev is not None and np.allclose(acc, prev, rtol=1e-3, atol=10.0):
            break
        prev = acc
    s_c, xt_c, l_c = acc[:, 0], acc[:, 1], acc[:, 2]

    w2, w1, wl = _weights()
    npad = qpc * slots - counts
    l_c = l_c - npad * math.log(12.0)             # pad rows: se = 12 exactly
    partial = (w1 * xt_c + w2 * s_c - wl * l_c).sum()
    loss = -partial / B
    return np.float32(loss)


if __name__ == "__main__":
    rng = np.random.default_rng(1)
    Bs = 4194304
    xs = rng.standard_normal((Bs, C)).astype(np.float32)
    ts = rng.integers(0, C, size=Bs).astype(np.int64)
    print("loss:", kernel(xs, ts))


# revision 19
# speedup vs baseline: 1.6241x; 1.0290x over previous
"""Trainium2 Bass kernel for nn_CELoss_Marginal_Smooth (CE loss with marginal
attention smoothing) on 8 NeuronCores.

Strategy
--------
loss = -mean_i[ (1-w2_i)*x[i,t_i] + w2_i*S_i - (1+11*w2_i)*lse_i ]
  where S_i = sum_c x[i,c], lse_i = log(sum_c exp(x[i,c])), and
  w2_i = (1-ALPHA)*att(t_i) takes one of 12 per-class values.

The host shards rows across 8 cores AND groups rows by target class inside
each core's shard (the loss is permutation-invariant, so row order is a
sharding/layout choice). Each (partition, class) cell is padded with zero
rows to a uniform count qpc. The staged per-core buffer is bf16 and
logit-plane-major within each class block: X[p, c, j, q] = x[row(p,c,q), j],
so every device-side operand is a contiguous bf16 slice:
  - exp            -> one ACT instruction per class group (the bottleneck:
                      1 elem/cycle/partition at 1.2 GHz, dtype-independent)
  - sumexp         -> DVE pairwise-add tree over contiguous bf16 planes
                      (2x packed mode; class planes adjacent in the free dim)
  - per-class sums -> PE ones-matmuls into per-class PSUM rows: S_c (whole
                      block), XT_c (own-logit plane), L_c (lse plane)
  - lse            -> one deferred ACT ln over the packed sumexp buffer
The device emits 12x3 raw per-class partials; the host applies the exact
fp64 class weights, corrects the known pad-row contribution (each pad row
adds exactly ln(12) to its L_c), and combines the 8 cores.
"""
import sys

if "/opt/trn_rl_repo" not in sys.path:
    sys.path.insert(0, "/opt/trn_rl_repo")

import math
from contextlib import ExitStack

import numpy as np

import concourse.bass as bass
import concourse.tile as tile
from concourse import bacc, mybir
from concourse.bass_utils import run_bass_kernel_spmd
from concourse.tile_rust import add_dep_helper

C = 12
P = 128
NCORES = 8
ALPHA = 0.6
MM_CHUNK = 512     # moving free-dim per rect matmul (PSUM bank width)

_F32 = mybir.dt.float32
_BF16 = mybir.dt.bfloat16
_FP8 = mybir.dt.float8e4
_AF = mybir.ActivationFunctionType

# group layout: (first class, n classes). The first two groups are single
# classes so the first exp only waits on a 1-class DMA; the last two are
# single classes to shorten the tail chain (exp -> tree -> ln -> out).
_GROUPS = [(0, 1), (1, 1), (2, 2), (4, 2), (6, 2), (8, 2), (10, 1), (11, 1)]
# plane-count chunks for the latency-critical head loads: ramp up transfer
# sizes so the first exp starts as early as possible without starving
_HEAD_CHUNKS = {0: (2, 4, 6), 1: (6, 6)}


def _att_values():
    i = np.arange(C)
    r, c = i // 4, i % 4
    up, dn = (r - 1 >= 0), (r + 1 <= 2)
    lf, rt = (c - 1 >= 0), (c + 1 <= 3)
    cnt = (up.astype(np.int32) + dn + lf + rt
           + (up & lf) + (up & rt) + (dn & lf) + (dn & rt))
    return 1.0 / cnt


def _weights():
    att = _att_values()
    w2 = (1.0 - ALPHA) * att          # weight of S_i
    w1 = 1.0 - w2                     # weight of x[i, t_i]
    wl = 1.0 + 11.0 * w2              # weight of lse_i
    return w2, w1, wl


def _build(qpc: int):
    """Build + finalize the per-core Bass program for a given qpc."""
    fp = qpc * C                      # free elements per class block
    nc = bacc.Bacc("TRN2", target_bir_lowering=False, debug=False,
                   num_devices=NCORES)
    # NOTE: the combined natural_log_exp_and_others activation table set
    # produces wrong Ln results on hardware (probed 2026-08-08); keep the
    # default per-function table selection (exp_and_others + natural_log).
    x = nc.declare_dram_parameter("x", [P, C * fp], _FP8, isOutput=False)
    out = nc.declare_dram_parameter("out", [C, 3], _F32, isOutput=True)

    with tile.TileContext(nc) as tc, ExitStack() as ctx:
        xp = ctx.enter_context(tc.tile_pool(name="xp", bufs=4))
        ep = ctx.enter_context(tc.tile_pool(name="ep", bufs=2))
        tp = ctx.enter_context(tc.tile_pool(name="tp", bufs=2))
        sp = ctx.enter_context(tc.tile_pool(name="sp", bufs=1))
        pp = ctx.enter_context(tc.tile_pool(name="pp", bufs=1, space="PSUM"))

        # matmul outputs must land at PSUM base partition 0, so class c's
        # partial sums are routed to PSUM row c via a one-hot stationary:
        # oneh[:, c*C + c] = 1, rest 0 -> out row c = column sums, rows
        # m != c accumulate zeros. One copy per matmul operand dtype.
        oneh8 = sp.tile([P, C * C], _FP8)
        onehb = sp.tile([P, C * C], _BF16)
        for oh in (oneh8, onehb):
            nc.vector.memset(oh[:], 0.0)
            ohv = oh[:].rearrange("p (a b) -> p a b", a=C)
            for c in range(C):
                nc.vector.memset(ohv[:, c, c:c + 1], 1.0)
        sebuf = sp.tile([P, C * qpc], _BF16)
        lsed = sp.tile([P, C * qpc], _BF16)
        ps_s = pp.tile([C, MM_CHUNK], _F32)
        ps_xt = pp.tile([C, MM_CHUNK], _F32)
        ps_l = pp.tile([C, MM_CHUNK], _F32)

        last_exp = None
        for c0, ng in _GROUPS:
            gf = ng * fp
            xt = xp.tile([P, gf], _FP8, tag="x")
            et = ep.tile([P, gf], _BF16, tag="e")
            if c0 in _HEAD_CHUNKS:
                # split the head loads so the exp pipeline starts as soon as
                # a small chunk lands (DMA fixed latency dominates)
                off = 0
                for planes in _HEAD_CHUNKS[c0]:
                    h = planes * qpc
                    nc.sync.dma_start(xt[:, off:off + h],
                                      x[:, c0 * fp + off:c0 * fp + off + h])
                    last_exp = nc.scalar.activation(et[:, off:off + h],
                                                    xt[:, off:off + h],
                                                    _AF.Exp)
                    off += h
                assert off == gf
            else:
                nc.sync.dma_start(xt[:], x[:, c0 * fp:c0 * fp + gf])
                last_exp = nc.scalar.activation(et[:], xt[:], _AF.Exp)

            # pairwise-add tree over the 12 logit planes of each class in
            # the group; all operands are contiguous bf16 runs of qpc (2x
            # packed DVE mode)
            ev = et[:].rearrange("p (t j q) -> p t j q", t=ng, j=C)
            t6 = tp.tile([P, ng, 6, qpc], _BF16, tag="t6")
            nc.vector.tensor_add(t6[:], ev[:, :, 0:6], ev[:, :, 6:12])
            t3 = tp.tile([P, ng, 3, qpc], _BF16, tag="t3")
            nc.vector.tensor_add(t3[:], t6[:, :, 0:3], t6[:, :, 3:6])
            t1 = tp.tile([P, ng, 1, qpc], _BF16, tag="t1")
            nc.vector.tensor_add(t1[:], t3[:, :, 0:1], t3[:, :, 1:2])
            sev = sebuf[:, c0 * qpc:(c0 + ng) * qpc].rearrange(
                "p (t j q) -> p t j q", t=ng, j=1)
            nc.vector.tensor_add(sev, t1[:], t3[:, :, 2:3])

            for u in range(ng):
                c = c0 + u
                lh = oneh8[:, c * C:(c + 1) * C]
                # S_c: sum of the whole class block, accumulated column-wise
                xflat = xt[:, u * fp:(u + 1) * fp]
                for i in range(0, fp, MM_CHUNK):
                    w = min(MM_CHUNK, fp - i)
                    nc.tensor.matmul(ps_s[:, 0:w], lhsT=lh,
                                     rhs=xflat[:, i:i + w],
                                     start=(c == 0 and i == 0),
                                     stop=(c == C - 1 and i + MM_CHUNK >= fp))
                # XT_c: sum of the own-logit plane
                xplane = xt[:, u * fp + c * qpc:u * fp + (c + 1) * qpc]
                nc.tensor.matmul(ps_xt[:, 0:qpc], lhsT=lh, rhs=xplane,
                                 start=(c == 0), stop=(c == C - 1))

        fin = sp.tile([C, 3], _F32)
        # S/XT accumulation groups close with class 11's matmuls, which only
        # depend on the last DMA -- these reduces overlap the ln stream
        nc.vector.tensor_reduce(fin[:, 0:1], ps_s[:, :],
                                axis=mybir.AxisListType.X,
                                op=mybir.AluOpType.add)
        nc.vector.tensor_reduce(fin[:, 1:2], ps_xt[:, 0:qpc],
                                axis=mybir.AxisListType.X,
                                op=mybir.AluOpType.add)

        # deferred: lse = ln(sumexp). Two ACT passes: the bulk (classes
        # 0..9, whose trees completed long before the last exp) and a short
        # tail pass for the last two classes, so the final ln only waits on
        # the last group's tree.
        for lo_c, hi_c in ((0, C - 2), (C - 2, C)):
            lo, hi = lo_c * qpc, hi_c * qpc
            ln_inst = nc.scalar.activation(lsed[:, lo:hi], sebuf[:, lo:hi],
                                           _AF.Ln)
            add_dep_helper(ln_inst.ins, last_exp.ins, False,
                           "ln after all exps (act stream ordering)")
            for c in range(lo_c, hi_c):
                nc.tensor.matmul(ps_l[:, 0:qpc],
                                 lhsT=onehb[:, c * C:(c + 1) * C],
                                 rhs=lsed[:, c * qpc:(c + 1) * qpc],
                                 start=(c == 0), stop=(c == C - 1))
        nc.vector.tensor_reduce(fin[:, 2:3], ps_l[:, 0:qpc],
                                axis=mybir.AxisListType.X,
                                op=mybir.AluOpType.add)
        nc.sync.dma_start(out[:], fin[:])
    nc.finalize()
    return nc


_PROG_CACHE: dict = {}
_LAST_IN_MAPS = None


def _program(qpc: int):
    if qpc not in _PROG_CACHE:
        _PROG_CACHE[qpc] = _build(qpc)
    return _PROG_CACHE[qpc]


def kernel(outputs: np.ndarray, targets: np.ndarray) -> np.ndarray:
    x = np.asarray(outputs)
    t = np.asarray(targets).astype(np.int64, copy=False).ravel()
    B = x.shape[0]
    assert x.shape == (B, C)

    counts = np.bincount(t, minlength=C)
    slots = NCORES * P
    # uniform per-(partition, class) row count; multiple of 8 keeps every
    # staged plane 4-byte aligned in the free dim
    qpc = max(64, 8 * math.ceil(counts.max() / (slots * 8)))

    # class-major index layout: A[c, s, q] = global row (or -1 pad)
    A = np.full((C, slots * qpc), -1, dtype=np.int64)
    order = np.argsort(t, kind="stable")
    bounds = np.concatenate(([0], np.cumsum(counts)))
    for c in range(C):
        A[c, :counts[c]] = order[bounds[c]:bounds[c + 1]]
    A = A.reshape(C, slots, qpc)

    xb = x.astype(mybir.dt.np(_FP8))
    in_maps = []
    for k in range(NCORES):
        idx = A[:, k * P:(k + 1) * P, :]          # [C, P, qpc]
        g = xb[idx.clip(min=0)]                   # [C, P, qpc, 12]
        g[idx < 0] = 0
        # plane-major within each class block: [p, c, j, q]
        xk = np.ascontiguousarray(g.transpose(1, 0, 3, 2)).reshape(P, -1)
        in_maps.append({"x": xk})

    nc = _program(qpc)
    global _LAST_IN_MAPS
    _LAST_IN_MAPS = in_maps

    # guard against transient execution flakes: rerun until two consecutive
    # runs agree on the combined partials
    prev = None
    for _ in range(4):
        res = run_bass_kernel_spmd(nc, in_maps, list(range(NCORES)))
        acc = np.zeros((C, 3), dtype=np.float64)
        for k in range(NCORES):
            acc += np.asarray(res.results[k]["out"]).astype(np.float64)
        if prev is not None and np.allclose(acc, prev, rtol=1e-3, atol=10.0):
            break
        prev = acc
    s_c, xt_c, l_c = acc[:, 0], acc[:, 1], acc[:, 2]

    w2, w1, wl = _weights()
    npad = qpc * slots - counts
    l_c = l_c - npad * math.log(12.0)             # pad rows: se = 12 exactly
    partial = (w1 * xt_c + w2 * s_c - wl * l_c).sum()
    loss = -partial / B
    return np.float32(loss)


if __name__ == "__main__":
    rng = np.random.default_rng(1)
    Bs = 4194304
    xs = rng.standard_normal((Bs, C)).astype(np.float32)
    ts = rng.integers(0, C, size=Bs).astype(np.int64)
    print("loss:", kernel(xs, ts))
